# revision 14
# baseline (speedup 1.0000x reference)
"""AttnBlock6 Trainium2 Bass kernel.

GroupNorm -> qkv 1x1conv -> patch-local attention + pooled global attention
-> combine -> proj -> residual, for x [2, 64, 448, 448] f32.

Sharding: 8 cores = 2 samples x 4 row-slabs of 112 rows each. Per-sample
coupling (GN stats, patch score matrix, pooled global k/v) is handled with
three tiny collectives over 4-core replica groups. Patch chunks (196 px,
row-major) and 8x8 pool blocks align exactly with slab boundaries.

On-chip layout: each slab is "stacked" as [128, 25088] — partition p = h*64+c
holds channel c of half-slab h (56 rows). The host pre-packs x into this
layout in bf16 (tolerance is 2e-2; bf16 everywhere on-chip), and unpacks the
bf16 output. GroupNorm is folded into the projection weights/biases, and the
output 1x1 conv is folded into the v projection (pv = (P @ Wv~) x + P @ bv~),
so patch attention directly produces proj(0.75*hp).

Self-contained: accepts FULL inputs, returns FULL output. The compiled
program and jitted executor are cached at module level (built on first use).
"""
import numpy as np

from concourse import bacc, mybir, tile, masks
from concourse import bass_utils

C = 64
P2 = 196                 # tokens per patch chunk
ROWS = 112               # image rows per core slab
SIZE = 448
HALF = 56 * 448          # 25088 = free-dim columns per half-slab
NCH = 128
PG_LOC = 784             # local pooled positions per core (14*56)
PG = 3136                # global pooled positions (56*56)
EPS = 1e-5
N_TOT = float(C * SIZE * SIZE)

RG = [[0, 1, 2, 3], [4, 5, 6, 7]]

F32 = mybir.dt.float32
BF16 = mybir.dt.bfloat16
AF = mybir.ActivationFunctionType
AXX = mybir.AxisListType.X

# phase-4 region ci becomes ready after patch chunk T4[ci] completes
T4 = {18: 0, 36: 1, 54: 2, 73: 3, 91: 4, 109: 5, 127: 6}


def build():
    nc = bacc.Bacc("TRN2", target_bir_lowering=False, debug=False, num_devices=8)

    x_in = nc.dram_tensor("x", [NCH, HALF], BF16, kind="ExternalInput")
    gn_w = nc.dram_tensor("gn_w", [C], F32, kind="ExternalInput")
    gn_b = nc.dram_tensor("gn_b", [C], F32, kind="ExternalInput")
    q_w = nc.dram_tensor("q_w", [C, C], F32, kind="ExternalInput")
    q_b = nc.dram_tensor("q_b", [C], F32, kind="ExternalInput")
    k_w = nc.dram_tensor("k_w", [C, C], F32, kind="ExternalInput")
    k_b = nc.dram_tensor("k_b", [C], F32, kind="ExternalInput")
    v_w = nc.dram_tensor("v_w", [C, C], F32, kind="ExternalInput")
    v_b = nc.dram_tensor("v_b", [C], F32, kind="ExternalInput")
    proj_w = nc.dram_tensor("proj_w", [C, C], F32, kind="ExternalInput")
    out_t = nc.dram_tensor("out", [NCH, HALF], BF16, kind="ExternalOutput")

    st_in = nc.dram_tensor("st_in", [NCH, 2], F32, kind="Internal")
    st_out = nc.dram_tensor("st_out", [NCH, 2], F32, kind="Internal")
    kv_in = nc.dram_tensor("kv_in", [2, C, PG_LOC], BF16, kind="Internal")
    kv_out = nc.dram_tensor("kv_out", [4, 2, C, PG_LOC], BF16, kind="Internal")
    wm_in = nc.dram_tensor("wm_in", [P2, P2], F32, kind="Internal")
    wm_out = nc.dram_tensor("wm_out", [P2, P2], F32, kind="Internal")

    wb = {"q": (q_w, q_b), "k": (k_w, k_b), "v": (v_w, v_b)}

    with tile.TileContext(nc) as tc:
        with tc.tile_pool(name="persist", bufs=1) as pp:
            # ---------------- constants ----------------
            id_bf = pp.tile([128, 128], BF16)
            masks.make_identity(nc, id_bf[:])
            ones_f = pp.tile([128, 128], F32)
            nc.gpsimd.memset(ones_f[:], 1.0)
            ones_bf = pp.tile([128, C], BF16)
            nc.gpsimd.memset(ones_bf[:], 1.0)

            bias_dup = {}
            for nm in ("q", "k", "v"):
                bt = pp.tile([NCH, 1], F32, name=f"bias_{nm}")
                nc.sync.dma_start(bt[0:C, :], wb[nm][1].ap().unsqueeze(1))
                nc.sync.dma_start(bt[C:NCH, :], wb[nm][1].ap().unsqueeze(1))
                bias_dup[nm] = bt
            gnw2 = pp.tile([NCH, 1], F32)
            nc.sync.dma_start(gnw2[0:C, :], gn_w.ap().unsqueeze(1))
            nc.sync.dma_start(gnw2[C:NCH, :], gn_w.ap().unsqueeze(1))
            gnb2 = pp.tile([NCH, 1], F32)
            nc.sync.dma_start(gnb2[0:C, :], gn_b.ap().unsqueeze(1))
            nc.sync.dma_start(gnb2[C:NCH, :], gn_b.ap().unsqueeze(1))

            # ---------------- persistent tensors ----------------
            X = pp.tile([NCH, HALF], BF16, name="Xbig")
            V = pp.tile([NCH, HALF], BF16, name="Vbig")
            px = pp.tile([NCH, 392], F32, name="px")
            ssq = pp.tile([NCH, 7], F32)
            wmT_hi = pp.tile([128, P2], BF16)
            wmT_lo = pp.tile([68, P2], BF16)
            kgf = pp.tile([C, PG], BF16)
            vgf = pp.tile([C, PG], BF16)
            vgT = pp.tile([128, 25 * C], BF16)
            qg_bf = pp.tile([C, PG_LOC], BF16)
            phg = pp.tile([NCH, 392], BF16, name="phg")

            # ============ phase 1: load + GN stats + 8x8 pool ============
            # x loads split across both HWDGE rings (sync + scalar) and
            # across partition halves for parallelism.
            HC = HALF // 2
            nc.sync.dma_start(X[0:C, 0:HC], x_in.ap()[0:C, 0:HC])
            nc.scalar.dma_start(X[C:NCH, 0:HC], x_in.ap()[C:NCH, 0:HC])
            nc.sync.dma_start(X[0:C, HC:HALF], x_in.ap()[0:C, HC:HALF])
            nc.scalar.dma_start(X[C:NCH, HC:HALF], x_in.ap()[C:NCH, HC:HALF])

            with tc.tile_pool(name="p1b", bufs=2) as p1b:
                for ci in range(7):
                    cols = slice(ci * 3584, (ci + 1) * 3584)
                    sq = p1b.tile([NCH, 3584], BF16, tag="sq")
                    nc.scalar.activation(sq[:], X[:, cols], AF.Square,
                                         accum_out=ssq[:, ci:ci + 1])
                    pl1 = p1b.tile([NCH, 448], F32, tag="pl1")
                    nc.vector.reduce_sum(
                        pl1[:].rearrange("p (r pc) -> p r pc", r=8),
                        X[:, cols].rearrange("p (r pc cc) -> p r pc cc",
                                             r=8, pc=56, cc=8),
                        axis=AXX,
                    )
                    nc.vector.reduce_sum(
                        px[:, ci * 56:(ci + 1) * 56],
                        pl1[:].rearrange("p (r pc) -> p pc r", r=8, pc=56),
                        axis=AXX,
                    )

            # ============ GN stats allreduce + weight/bias folding ============
            blk = {}
            btld = {}
            blk_pv = pp.tile([NCH, NCH], BF16)
            b_pv = pp.tile([NCH, 1], F32)
            pT64 = pp.tile([C, C], BF16)
            with tc.tile_pool(name="gn", bufs=1) as gp, \
                 tc.tile_pool(name="gnp", bufs=1, space="PSUM") as gpp:
                # stage f32 weights (transposed) and build block-diagonal forms
                wts = {}
                for nm, wd in (("q", q_w), ("k", k_w), ("v", v_w), ("p", proj_w)):
                    wt = gp.tile([NCH, C], F32, name=f"wt_{nm}")
                    nc.sync.dma_start(wt[0:C, :], wd.ap().transpose([1, 0]))
                    nc.sync.dma_start(wt[C:NCH, :], wd.ap().transpose([1, 0]))
                    wts[nm] = wt
                vw_raw = gp.tile([C, C], BF16)   # v_w untransposed [m, c]
                vw_stage = gp.tile([C, C], F32)
                nc.sync.dma_start(vw_stage[:], v_w.ap())
                nc.scalar.copy(vw_raw[:], vw_stage[:])
                blkraw = {}
                for nm in ("q", "k", "v"):
                    b = gp.tile([NCH, NCH], BF16, name=f"blkraw_{nm}")
                    nc.gpsimd.memset(b[:], 0.0)
                    nc.scalar.copy(b[0:C, 0:C], wts[nm][0:C, :])
                    nc.scalar.copy(b[C:NCH, C:NCH], wts[nm][C:NCH, :])
                    blkraw[nm] = b
                blk_p = gp.tile([NCH, NCH], BF16)
                nc.gpsimd.memset(blk_p[:], 0.0)
                nc.scalar.copy(blk_p[0:C, 0:C], wts["p"][0:C, :])
                nc.scalar.copy(blk_p[C:NCH, C:NCH], wts["p"][C:NCH, :])
                nc.scalar.copy(pT64[:], wts["p"][0:C, :])

                # stats: slab sum comes free from the pooled sums
                st2 = gp.tile([NCH, 2], F32)
                nc.vector.reduce_sum(st2[:, 0:1], px[:], axis=AXX)
                nc.vector.reduce_sum(st2[:, 1:2], ssq[:], axis=AXX)
                st_ps = gpp.tile([NCH, 2], F32)
                nc.tensor.matmul(st_ps[:], ones_f[:], st2[:], start=True, stop=True)
                st_sb = gp.tile([NCH, 2], F32)
                nc.scalar.copy(st_sb[:], st_ps[:])
                nc.sync.dma_start(st_in.ap(), st_sb[:])
                nc.gpsimd.collective_compute(
                    "AllReduce", mybir.AluOpType.add, replica_groups=RG,
                    ins=[st_in.ap()], outs=[st_out.ap()],
                )
                gst = gp.tile([NCH, 2], F32)
                nc.sync.dma_start(gst[:], st_out.ap())

                mean_t = gp.tile([NCH, 1], F32)
                nc.scalar.mul(mean_t[:], gst[:, 0:1], 1.0 / N_TOT)
                ex2_t = gp.tile([NCH, 1], F32)
                nc.scalar.mul(ex2_t[:], gst[:, 1:2], 1.0 / N_TOT)
                msq_t = gp.tile([NCH, 1], F32)
                nc.scalar.square(msq_t[:], mean_t[:])
                var_t = gp.tile([NCH, 1], F32)
                nc.vector.tensor_sub(var_t[:], ex2_t[:], msq_t[:])
                eps_t = gp.tile([NCH, 1], F32)
                nc.gpsimd.memset(eps_t[:], EPS)
                sd_t = gp.tile([NCH, 1], F32)
                nc.scalar.activation(sd_t[:], var_t[:], AF.Sqrt, bias=eps_t[:])
                rstd_t = gp.tile([NCH, 1], F32)
                nc.vector.reciprocal(rstd_t[:], sd_t[:])

                scale_t = gp.tile([NCH, 1], F32)
                nc.vector.tensor_mul(scale_t[:], rstd_t[:], gnw2[:])
                tmp_t = gp.tile([NCH, 1], F32)
                nc.vector.tensor_mul(tmp_t[:], mean_t[:], scale_t[:])
                gb_t = gp.tile([NCH, 1], F32)
                nc.vector.tensor_sub(gb_t[:], gnb2[:], tmp_t[:])
                gb_bf = gp.tile([NCH, 1], BF16)
                nc.scalar.copy(gb_bf[:], gb_t[:])
                scale64_t = gp.tile([NCH, 1], F32)
                nc.scalar.mul(scale64_t[:], scale_t[:], 1.0 / 64.0)

                for nm in ("q", "k"):
                    wf = gp.tile([NCH, C], F32, name=f"wf_{nm}")
                    nc.vector.tensor_scalar_mul(wf[:], wts[nm][:], scale_t[:])
                    b = pp.tile([NCH, NCH], BF16, name=f"blk_{nm}")
                    nc.gpsimd.memset(b[:], 0.0)
                    nc.scalar.copy(b[0:C, 0:C], wf[0:C, :])
                    nc.scalar.copy(b[C:NCH, C:NCH], wf[C:NCH, :])
                    blk[nm] = b

                for nm in ("q", "k", "v"):
                    bps = gpp.tile([NCH, 1], F32, tag="bps", bufs=2)
                    nc.tensor.matmul(bps[:], blkraw[nm][:], gb_bf[:],
                                     start=True, stop=True)
                    bt = gp.tile([NCH, 1], F32, name=f"btld_{nm}")
                    nc.scalar.activation(bt[:], bps[:], AF.Identity,
                                         bias=bias_dup[nm][:])
                    btld[nm] = bt
                btld["q"] = btld["q"]; btld["k"] = btld["k"]
                btq = pp.tile([NCH, 1], F32)
                nc.scalar.copy(btq[:], btld["q"][:])
                btk = pp.tile([NCH, 1], F32)
                nc.scalar.copy(btk[:], btld["k"][:])

                # pv = (P @ Wv~) x + P @ bv~   (proj folded into v path)
                pw_ps = gpp.tile([C, C], F32, tag="bps", bufs=2)
                nc.tensor.matmul(pw_ps[:], vw_raw[:], pT64[:], start=True, stop=True)
                pwf = gp.tile([C, C], F32)
                nc.vector.tensor_scalar_mul(pwf[:], pw_ps[:], scale_t[0:C, :])
                pw_sb = gp.tile([C, C], BF16)
                nc.scalar.copy(pw_sb[:], pwf[:])
                nc.gpsimd.memset(blk_pv[:], 0.0)
                nc.sync.dma_start(blk_pv[0:C, 0:C], pw_sb[:])
                nc.sync.dma_start(blk_pv[C:NCH, C:NCH], pw_sb[:])
                btv_bf = gp.tile([NCH, 1], BF16)
                nc.scalar.copy(btv_bf[:], btld["v"][:])
                bpv_ps = gpp.tile([NCH, 1], F32, tag="bps", bufs=2)
                nc.tensor.matmul(bpv_ps[:], blk_p[:], btv_bf[:], start=True, stop=True)
                nc.scalar.copy(b_pv[:], bpv_ps[:])

                # ---- pooled projections qg/kg/vg from pooled xn ----
                pxn = gp.tile([NCH, 392], BF16)
                nc.vector.tensor_scalar(
                    pxn[:], px[:], scale64_t[:], gb_t[:],
                    op0=mybir.AluOpType.mult, op1=mybir.AluOpType.add,
                )
                pxu = gp.tile([C, PG_LOC], BF16)
                nc.sync.dma_start(pxu[:, 0:392], pxn[0:C, :])
                nc.sync.dma_start(pxu[:, 392:784], pxn[C:NCH, :])

                qb8 = gp.tile([NCH, 1], F32)
                nc.scalar.mul(qb8[:], bias_dup["q"][:], 0.125)
                wT64 = {}
                for nm in ("q", "k", "v"):
                    w64 = gp.tile([C, C], BF16, name=f"w64_{nm}")
                    nc.scalar.copy(w64[:], wts[nm][0:C, :])
                    wT64[nm] = w64
                kg_loc = gp.tile([C, PG_LOC], BF16)
                vg_loc = gp.tile([C, PG_LOC], BF16)
                for s in range(2):
                    cs = slice(s * 392, (s + 1) * 392)
                    qps = gpp.tile([C, 392], F32, tag="pool_ps", bufs=4)
                    nc.tensor.matmul(qps[:], wT64["q"][:], pxu[:, cs],
                                     start=True, stop=True)
                    nc.scalar.activation(qg_bf[:, cs], qps[:], AF.Identity,
                                         bias=qb8[0:C, :], scale=0.125)
                    kps = gpp.tile([C, 392], F32, tag="pool_ps", bufs=4)
                    nc.tensor.matmul(kps[:], wT64["k"][:], pxu[:, cs],
                                     start=True, stop=True)
                    nc.scalar.activation(kg_loc[:, cs], kps[:], AF.Identity,
                                         bias=bias_dup["k"][0:C, :])
                    vps = gpp.tile([C, 392], F32, tag="pool_ps", bufs=4)
                    nc.tensor.matmul(vps[:], wT64["v"][:], pxu[:, cs],
                                     start=True, stop=True)
                    nc.scalar.activation(vg_loc[:, cs], vps[:], AF.Identity,
                                         bias=bias_dup["v"][0:C, :])
                nc.sync.dma_start(kv_in.ap()[0], kg_loc[:])
                nc.sync.dma_start(kv_in.ap()[1], vg_loc[:])
                nc.gpsimd.collective_compute(
                    "AllGather", mybir.AluOpType.bypass, replica_groups=RG,
                    ins=[kv_in.ap()], outs=[kv_out.ap()],
                )
                nc.sync.dma_start(
                    kgf[:].rearrange("c (r f) -> c r f", r=4),
                    kv_out.ap()[:, 0].transpose([1, 0, 2]),
                )
                nc.sync.dma_start(
                    vgf[:].rearrange("c (r f) -> c r f", r=4),
                    kv_out.ap()[:, 1].transpose([1, 0, 2]),
                )

            # vg transposed into [q(part), c] chunks
            with tc.tile_pool(name="vgt_ps", bufs=4, space="PSUM") as vtp:
                for qc in range(25):
                    w = min(128, PG - qc * 128)
                    tp = vtp.tile([128, C], BF16, tag="vgt")
                    nc.tensor.transpose(tp[0:w, :], vgf[:, qc * 128:qc * 128 + w],
                                        id_bf[0:C, 0:C])
                    nc.vector.tensor_copy(vgT[0:w, qc * C:(qc + 1) * C], tp[0:w, :])

            # ============ phase 2: q/k/pv + patch score accumulation ============
            with tc.tile_pool(name="p2ps", bufs=2, space="PSUM") as p2ps, \
                 tc.tile_pool(name="wmps", bufs=1, space="PSUM") as wmps, \
                 tc.tile_pool(name="p2sb", bufs=3) as p2sb:
                wmA = wmps.tile([128, P2], F32)
                wmB = wmps.tile([68, P2], F32)
                for u in range(64):
                    cols = slice(u * 392, (u + 1) * 392)
                    qps = p2ps.tile([NCH, 392], F32, tag="qps")
                    nc.tensor.matmul(qps[:], blk["q"][:], X[:, cols],
                                     start=True, stop=True)
                    q_sb = p2sb.tile([NCH, 392], BF16, tag="q_sb")
                    nc.scalar.activation(q_sb[:], qps[:], AF.Identity, bias=btq[:])
                    kps = p2ps.tile([NCH, 392], F32, tag="kps")
                    nc.tensor.matmul(kps[:], blk["k"][:], X[:, cols],
                                     start=True, stop=True)
                    k_sb = p2sb.tile([NCH, 392], BF16, tag="k_sb")
                    nc.scalar.activation(k_sb[:], kps[:], AF.Identity, bias=btk[:])
                    vps = p2ps.tile([NCH, 392], F32, tag="vps")
                    nc.tensor.matmul(vps[:], blk_pv[:], X[:, cols],
                                     start=True, stop=True)
                    nc.vector.tensor_scalar_add(V[:, cols], vps[:], b_pv[:])
                    for sub in range(2):
                        a = slice(sub * P2, sub * P2 + 128)
                        bsl = slice(sub * P2 + 128, (sub + 1) * P2)
                        ks = k_sb[:, sub * P2:(sub + 1) * P2]
                        first = (u == 0 and sub == 0)
                        last = (u == 63 and sub == 1)
                        nc.tensor.matmul(wmA[:], q_sb[:, a], ks,
                                         start=first, stop=last)
                        nc.tensor.matmul(wmB[:], q_sb[:, bsl], ks,
                                         start=first, stop=last)
                wmA_sb = p2sb.tile([128, P2], F32, tag="wmA_sb", bufs=1)
                nc.vector.tensor_copy(wmA_sb[:], wmA[:])
                wmB_sb = p2sb.tile([68, P2], F32, tag="wmB_sb", bufs=1)
                nc.vector.tensor_copy(wmB_sb[:], wmB[:])
                nc.sync.dma_start(wm_in.ap()[0:128], wmA_sb[:])
                nc.sync.dma_start(wm_in.ap()[128:P2], wmB_sb[:])

            nc.gpsimd.collective_compute(
                "AllReduce", mybir.AluOpType.add, replica_groups=RG,
                ins=[wm_in.ap()], outs=[wm_out.ap()],
            )

            # ============ phase G: global pooled attention ============
            # (emitted here so it overlaps the wm AllReduce latency)
            with tc.tile_pool(name="hgps", bufs=1, space="PSUM") as hgp:
                hg_ps = [hgp.tile([C, 392], F32, name=f"hg{s}") for s in range(2)]
                rcp_sb = None
                with tc.tile_pool(name="g1", bufs=3) as g1, \
                     tc.tile_pool(name="g1ps", bufs=4, space="PSUM") as g1ps, \
                     tc.tile_pool(name="dnps", bufs=1, space="PSUM") as dnps:
                    dn = [dnps.tile([1, 392], F32, name=f"dn{s}") for s in range(2)]
                    for qc in range(25):
                        w = min(128, PG - qc * 128)
                        e_ch = g1.tile([128, PG_LOC], BF16, tag="e_ch")
                        for s in range(2):
                            cs = slice(s * 392, (s + 1) * 392)
                            sT = g1ps.tile([128, 392], F32, tag="sT")
                            nc.tensor.matmul(sT[0:w, :],
                                             kgf[:, qc * 128:qc * 128 + w],
                                             qg_bf[:, cs], start=True, stop=True)
                            nc.scalar.activation(e_ch[0:w, cs], sT[0:w, :], AF.Exp)
                            nc.tensor.matmul(dn[s][:], ones_bf[0:w, 0:1],
                                             e_ch[0:w, cs],
                                             start=(qc == 0), stop=(qc == 24))
                            nc.tensor.matmul(hg_ps[s][:],
                                             vgT[0:w, qc * C:(qc + 1) * C],
                                             e_ch[0:w, cs],
                                             start=(qc == 0), stop=(qc == 24))
                    rcp_sb = g1.tile([1, PG_LOC], F32, tag="rcp", bufs=1)
                    nc.vector.reciprocal(rcp_sb[:, 0:392], dn[0][:])
                    nc.vector.reciprocal(rcp_sb[:, 392:784], dn[1][:])

                with tc.tile_pool(name="g2", bufs=1) as g2, \
                     tc.tile_pool(name="g2ps", bufs=1, space="PSUM") as g2ps:
                    rcb = g2.tile([1, PG_LOC], BF16)
                    nc.scalar.copy(rcb[:], rcp_sb[:])
                    hg_n = g2.tile([C, PG_LOC], BF16)
                    phg_sc = g2.tile([C, PG_LOC], BF16)
                    for s in range(2):
                        cs = slice(s * 392, (s + 1) * 392)
                        rcp128 = g2ps.tile([C, 392], F32, tag="rcp128", bufs=2)
                        nc.tensor.matmul(rcp128[:], ones_bf[0:1, :],
                                         rcb[:, cs], start=True, stop=True)
                        rcp128_sb = g2.tile([C, 392], BF16, tag="rcp128_sb", bufs=2)
                        nc.scalar.copy(rcp128_sb[:], rcp128[:])
                        nc.vector.tensor_mul(hg_n[:, cs], hg_ps[s][:], rcp128_sb[:])
                    for s in range(2):
                        cs = slice(s * 392, (s + 1) * 392)
                        phg_ps = g2ps.tile([C, 392], F32, tag="phg_ps", bufs=2)
                        nc.tensor.matmul(phg_ps[:], pT64[:], hg_n[:, cs],
                                         start=True, stop=True)
                        nc.vector.tensor_scalar_mul(phg_sc[:, cs], phg_ps[:], 0.25)
                    nc.sync.dma_start(phg[0:C, :], phg_sc[:, 0:392])
                    nc.sync.dma_start(phg[C:NCH, :], phg_sc[:, 392:784])

            # ============ wm softmax + transpose (after AllReduce) ============
            with tc.tile_pool(name="wmsb", bufs=1) as wsb, \
                 tc.tile_pool(name="wmtp", bufs=4, space="PSUM") as wtp:
                wmn = {}
                for nm, rows in (("hi", slice(0, 128)), ("lo", slice(128, P2))):
                    n = rows.stop - rows.start
                    wt = wsb.tile([n, P2], F32, name=f"wmr_{nm}")
                    nc.sync.dma_start(wt[:], wm_out.ap()[rows])
                    mx = wsb.tile([n, 1], F32, name=f"mx_{nm}")
                    nc.vector.reduce_max(mx[:], wt[:], axis=AXX)
                    mxn = wsb.tile([n, 1], F32, name=f"mxn_{nm}")
                    nc.vector.tensor_scalar_mul(mxn[:], mx[:], -1.0 / 256.0)
                    we = wsb.tile([n, P2], F32, name=f"we_{nm}")
                    rs = wsb.tile([n, 1], F32, name=f"rs_{nm}")
                    nc.scalar.activation(we[:], wt[:], AF.Exp, bias=mxn[:],
                                         scale=1.0 / 256.0, accum_out=rs[:])
                    rc = wsb.tile([n, 1], F32, name=f"rc_{nm}")
                    nc.vector.reciprocal(rc[:], rs[:])
                    rc75 = wsb.tile([n, 1], F32, name=f"rc75_{nm}")
                    nc.scalar.mul(rc75[:], rc[:], 0.75)
                    wn = wsb.tile([n, P2], BF16, name=f"wmn_{nm}")
                    nc.vector.tensor_scalar_mul(wn[:], we[:], rc75[:])
                    wmn[nm] = wn
                # wmT[k, p] = wmn[p, k] in four pieces
                for src, csl, dst, dcols in (
                    ("hi", slice(0, 128), wmT_hi, slice(0, 128)),
                    ("hi", slice(128, P2), wmT_lo, slice(0, 128)),
                    ("lo", slice(0, 128), wmT_hi, slice(128, P2)),
                    ("lo", slice(128, P2), wmT_lo, slice(128, P2)),
                ):
                    inap = wmn[src][:, csl]
                    n_p = inap.shape[0]
                    n_k = csl.stop - csl.start
                    tp = wtp.tile([128, 128], BF16, tag="wmt_ps")
                    nc.tensor.transpose(tp[0:n_k, 0:n_p], inap, id_bf[0:n_p, 0:n_p])
                    nc.vector.tensor_copy(dst[0:n_k, dcols], tp[0:n_k, 0:n_p])

            # ====== phase 3: patch attention (+proj via pv) + residual ======
            # ====== phase 4 (interleaved): add upsampled global, store ======
            with tc.tile_pool(name="p3ps", bufs=2, space="PSUM") as p3ps, \
                 tc.tile_pool(name="p3sb", bufs=3) as p3sb:
                for t in range(128):
                    tcols = slice(t * P2, (t + 1) * P2)
                    ta = p3ps.tile([128, 128], BF16, tag="ta")
                    nc.tensor.transpose(ta[:], V[:, t * P2:t * P2 + 128], id_bf[:])
                    tb = p3ps.tile([68, 128], BF16, tag="tb")
                    nc.tensor.transpose(tb[:], V[:, t * P2 + 128:(t + 1) * P2],
                                        id_bf[:])
                    vta = p3sb.tile([128, 128], BF16, tag="vta")
                    nc.vector.tensor_copy(vta[:], ta[:])
                    vtb = p3sb.tile([68, 128], BF16, tag="vtb")
                    nc.vector.tensor_copy(vtb[:], tb[:])
                    hp_ps = p3ps.tile([128, P2], F32, tag="hp_ps")
                    nc.tensor.matmul(hp_ps[:], vta[:], wmT_hi[:],
                                     start=True, stop=False)
                    nc.tensor.matmul(hp_ps[:], vtb[:], wmT_lo[:],
                                     start=False, stop=True)
                    nc.vector.tensor_add(V[:, tcols], hp_ps[:], X[:, tcols])

                    ci = T4.get(t)
                    if ci is not None:
                        cols = slice(ci * 3584, (ci + 1) * 3584)
                        bcast = (phg[:, ci * 56:(ci + 1) * 56]
                                 .unsqueeze(1).unsqueeze(3)
                                 .broadcast_to([NCH, 8, 56, 8]))
                        v4 = V[:, cols].rearrange("p (r pc cc) -> p r pc cc",
                                                  r=8, pc=56, cc=8)
                        nc.vector.tensor_tensor(v4, v4, bcast,
                                                op=mybir.AluOpType.add)
                        eng = nc.sync if ci % 2 == 0 else nc.scalar
                        eng.dma_start(out_t.ap()[:, cols], V[:, cols])

    nc.compile()
    return nc


_NC = None


def _get_nc():
    global _NC
    if _NC is None:
        _NC = build()
    return _NC


def _pack_x(x):
    import ml_dtypes
    # [2,64,448,448] -> [8 cores, 128, 25088] stacked bf16
    xs = x.reshape(2, C, 4, 2, 56, SIZE).transpose(0, 2, 3, 1, 4, 5)
    return np.ascontiguousarray(xs.reshape(8, NCH, HALF)).astype(ml_dtypes.bfloat16)


def _unpack_out(o):
    # [8, 128, 25088] bf16 -> [2, 64, 448, 448] f32
    o = np.asarray(o, dtype=np.float32).reshape(2, 4, 2, C, 56, SIZE)
    return np.ascontiguousarray(o.transpose(0, 3, 1, 2, 4, 5)).reshape(
        2, C, SIZE, SIZE)


def kernel(x, gn_w, gn_b, q_w, q_b, k_w, k_b, v_w, v_b, proj_w):
    nc = _get_nc()
    x = np.asarray(x, dtype=np.float32)
    xp = _pack_x(x)
    small = {
        "gn_w": gn_w, "gn_b": gn_b, "q_w": q_w, "q_b": q_b,
        "k_w": k_w, "k_b": k_b, "v_w": v_w, "v_b": v_b, "proj_w": proj_w,
    }
    small = {k: np.ascontiguousarray(np.asarray(v, dtype=np.float32))
             for k, v in small.items()}
    in_maps = []
    for r in range(8):
        m = dict(small)
        m["x"] = xp[r]
        in_maps.append(m)
    res = bass_utils.run_bass_kernel_spmd(nc, in_maps, core_ids=list(range(8)))
    return _unpack_out(np.stack([res.results[r]["out"] for r in range(8)]))


if __name__ == "__main__":
    build()
    print("build+compile OK")


# revision 30
# speedup vs baseline: 1.3567x; 1.3567x over previous
"""AttnBlock6 Trainium2 Bass kernel.

GroupNorm -> qkv 1x1conv -> patch-local attention + pooled global attention
-> combine -> proj -> residual, for x [2, 64, 448, 448] f32.

Sharding: 8 cores = 2 samples x 4 row-slabs of 112 rows each. Per-sample
coupling (GN stats, patch score matrix, pooled global k/v) is handled with
three tiny collectives over 4-core replica groups. Patch chunks (196 px,
row-major) and 8x8 pool blocks align exactly with slab boundaries.

On-chip layout: each slab is "stacked" as [128, 25088] — partition p = h*64+c
holds channel c of half-slab h (56 rows). The host pre-packs x into this
layout in bf16 (tolerance is 2e-2; bf16 everywhere on-chip) and pre-transposes
the 64x64 weights; the bf16 output is unpacked host-side. GroupNorm is folded
into the projection weights/biases, and the output 1x1 conv is folded into
the v projection (pv = (P @ Wv~) x + P @ bv~), so patch attention directly
produces proj(0.75*hp). The global-pool AllGather ships raw pooled x so it
overlaps the stats AllReduce.

Self-contained: accepts FULL inputs, returns FULL output. The compiled
program is cached at module level; the first call compiles, later calls only
execute.
"""
import numpy as np

from concourse import bacc, mybir, tile, masks
from concourse import bass_utils

C = 64
P2 = 196                 # tokens per patch chunk
ROWS = 112               # image rows per core slab
SIZE = 448
HALF = 56 * 448          # 25088 = free-dim columns per half-slab
NCH = 128
PG_LOC = 784             # local pooled positions per core (14*56)
PG = 3136                # global pooled positions (56*56)
EPS = 1e-5
N_TOT = float(C * SIZE * SIZE)

RG = [[0, 1, 2, 3], [4, 5, 6, 7]]

F32 = mybir.dt.float32
BF16 = mybir.dt.bfloat16
AF = mybir.ActivationFunctionType
AXX = mybir.AxisListType.X

# phase-4 region ci becomes ready after patch chunk T4[ci] completes
T4 = {18: 0, 36: 1, 54: 2, 73: 3, 91: 4, 109: 5, 127: 6}


def build():
    nc = bacc.Bacc("TRN2", target_bir_lowering=False, debug=False, num_devices=8)

    # NOTE: q_w/k_w/v_w/proj_w are fed PRE-TRANSPOSED by the host wrapper
    # (value = W.T, so [c_in, c_out] row-major — a contiguous DMA gives W^T
    # with c_in on partitions).
    x_in = nc.dram_tensor("x", [NCH, HALF], BF16, kind="ExternalInput")
    gn_w = nc.dram_tensor("gn_w", [C], F32, kind="ExternalInput")
    gn_b = nc.dram_tensor("gn_b", [C], F32, kind="ExternalInput")
    q_w = nc.dram_tensor("q_w", [C, C], F32, kind="ExternalInput")
    q_b = nc.dram_tensor("q_b", [C], F32, kind="ExternalInput")
    k_w = nc.dram_tensor("k_w", [C, C], F32, kind="ExternalInput")
    k_b = nc.dram_tensor("k_b", [C], F32, kind="ExternalInput")
    v_w = nc.dram_tensor("v_w", [C, C], F32, kind="ExternalInput")
    v_b = nc.dram_tensor("v_b", [C], F32, kind="ExternalInput")
    proj_w = nc.dram_tensor("proj_w", [C, C], F32, kind="ExternalInput")
    out_t = nc.dram_tensor("out", [NCH, HALF], BF16, kind="ExternalOutput")

    # px_in col 784 carries the core's [sum, sumsq] GN partials (partition 0)
    px_in = nc.dram_tensor("px_in", [C, PG_LOC + 2], F32, kind="Internal")
    px_out = nc.dram_tensor("px_out", [4, C, PG_LOC + 2], F32, kind="Internal")
    wm_in = nc.dram_tensor("wm_in", [P2, P2], F32, kind="Internal")
    wm_out = nc.dram_tensor("wm_out", [P2, P2], F32, kind="Internal")

    wb = {"q": (q_w, q_b), "k": (k_w, k_b), "v": (v_w, v_b)}

    with tile.TileContext(nc) as tc:
        with tc.tile_pool(name="persist", bufs=1) as pp:
            # ---------------- constants & weights (all DMAs contiguous) ----
            id_bf = pp.tile([128, 128], BF16)
            masks.make_identity(nc, id_bf[:])
            ones_f = pp.tile([128, 128], F32)
            nc.gpsimd.memset(ones_f[:], 1.0)
            ones_bf = pp.tile([128, C], BF16)
            nc.gpsimd.memset(ones_bf[:], 1.0)

            wts = {}
            for nm, wd in (("q", q_w), ("k", k_w), ("v", v_w), ("p", proj_w)):
                wt = pp.tile([NCH, C], F32, name=f"wt_{nm}")
                nc.sync.dma_start(wt[0:C, :], wd.ap())
                nc.sync.dma_start(wt[C:NCH, :], wd.ap())
                wts[nm] = wt
            bias_dup = {}
            for nm in ("q", "k", "v"):
                bt = pp.tile([NCH, 1], F32, name=f"bias_{nm}")
                nc.sync.dma_start(bt[0:C, :], wb[nm][1].ap().unsqueeze(1))
                nc.sync.dma_start(bt[C:NCH, :], wb[nm][1].ap().unsqueeze(1))
                bias_dup[nm] = bt
            gnw2 = pp.tile([NCH, 1], F32)
            nc.sync.dma_start(gnw2[0:C, :], gn_w.ap().unsqueeze(1))
            nc.sync.dma_start(gnw2[C:NCH, :], gn_w.ap().unsqueeze(1))
            gnb2 = pp.tile([NCH, 1], F32)
            nc.sync.dma_start(gnb2[0:C, :], gn_b.ap().unsqueeze(1))
            nc.sync.dma_start(gnb2[C:NCH, :], gn_b.ap().unsqueeze(1))

            # ---------------- persistent tensors ----------------
            X = pp.tile([NCH, HALF], BF16, name="Xbig")
            V = pp.tile([NCH, HALF], BF16, name="Vbig")
            px = pp.tile([NCH, 392], F32, name="px")
            ssq = pp.tile([NCH, 7], F32)
            wmT_hi = pp.tile([128, P2], BF16)
            wmT_lo = pp.tile([68, P2], BF16)
            kgf = pp.tile([C, PG], BF16)
            vgf = pp.tile([C, PG], BF16)
            vgT = pp.tile([128, 25 * (C + 1)], BF16)   # col 64 of each block = 1.0
            qg_bf = pp.tile([C, PG_LOC], BF16)
            phg = pp.tile([NCH, 392], BF16, name="phg")

            # ============ phase 1: load + GN stats + 8x8 pool ============
            # x loads: 8 pieces across 4 queues (2 HWDGE rings + 2 SWDGE).
            Q = HALF // 4
            engs = [nc.sync, nc.scalar, nc.gpsimd]
            for cq in range(4):
                csl = slice(cq * Q, (cq + 1) * Q)
                engs[(2 * cq) % 3].dma_start(X[0:C, csl], x_in.ap()[0:C, csl])
                engs[(2 * cq + 1) % 3].dma_start(X[C:NCH, csl], x_in.ap()[C:NCH, csl])

            with tc.tile_pool(name="p1b", bufs=2) as p1b:
                for ci in range(7):
                    cols = slice(ci * 3584, (ci + 1) * 3584)
                    sq = p1b.tile([NCH, 3584], BF16, tag="sq")
                    nc.scalar.activation(sq[:], X[:, cols], AF.Square,
                                         accum_out=ssq[:, ci:ci + 1])
                    pl1 = p1b.tile([NCH, 448], F32, tag="pl1")
                    nc.vector.reduce_sum(
                        pl1[:].rearrange("p (r pc) -> p r pc", r=8),
                        X[:, cols].rearrange("p (r pc cc) -> p r pc cc",
                                             r=8, pc=56, cc=8),
                        axis=AXX,
                    )
                    nc.vector.reduce_sum(
                        px[:, ci * 56:(ci + 1) * 56],
                        pl1[:].rearrange("p (r pc) -> p pc r", r=8, pc=56),
                        axis=AXX,
                    )

            # ============ stats AR + raw-pool AG (overlapped) ============
            blk = {}
            blk_pv = pp.tile([NCH, NCH], BF16)
            b_pv = pp.tile([NCH, 1], F32)
            pT64 = pp.tile([C, C], BF16)
            btq = pp.tile([NCH, 1], F32)
            btk = pp.tile([NCH, 1], F32)
            with tc.tile_pool(name="gn", bufs=1) as gp, \
                 tc.tile_pool(name="gnp", bufs=1, space="PSUM") as gpp:
                # local GN partials; shipped inside the px AllGather
                st2 = gp.tile([NCH, 2], F32)
                nc.vector.reduce_sum(st2[:, 0:1], px[:], axis=AXX)
                nc.vector.reduce_sum(st2[:, 1:2], ssq[:], axis=AXX)
                st_ps = gpp.tile([NCH, 2], F32)
                nc.tensor.matmul(st_ps[:], ones_f[:], st2[:], start=True, stop=True)
                st_sb = gp.tile([NCH, 2], F32)
                nc.scalar.copy(st_sb[:], st_ps[:])
                nc.sync.dma_start(px_in.ap()[:, 0:392], px[0:C, :])
                nc.sync.dma_start(px_in.ap()[:, 392:784], px[C:NCH, :])
                nc.sync.dma_start(px_in.ap()[:, PG_LOC:PG_LOC + 2],
                                  st_sb[0:C, :])
                nc.gpsimd.collective_compute(
                    "AllGather", mybir.AluOpType.bypass, replica_groups=RG,
                    ins=[px_in.ap()], outs=[px_out.ap()],
                )
                # combine the 4 gathered [sum, sumsq] pairs -> broadcast [128, 2]
                g4 = gp.tile([1, 8], F32)
                nc.sync.dma_start(g4[:].rearrange("p (r t) -> p r t", r=4),
                                  px_out.ap()[:, 0, PG_LOC:PG_LOC + 2])
                s2 = gp.tile([1, 2], F32)
                nc.vector.reduce_sum(
                    s2[:], g4[:].rearrange("p (r t) -> p t r", r=4), axis=AXX)
                gst_ps = gpp.tile([NCH, 2], F32)
                nc.tensor.matmul(gst_ps[:], ones_f[0:1, :], s2[:],
                                 start=True, stop=True)
                gst = gp.tile([NCH, 2], F32)
                nc.scalar.copy(gst[:], gst_ps[:])

                mean_t = gp.tile([NCH, 1], F32)
                nc.scalar.mul(mean_t[:], gst[:, 0:1], 1.0 / N_TOT)
                ex2_t = gp.tile([NCH, 1], F32)
                nc.scalar.mul(ex2_t[:], gst[:, 1:2], 1.0 / N_TOT)
                msq_t = gp.tile([NCH, 1], F32)
                nc.scalar.square(msq_t[:], mean_t[:])
                var_t = gp.tile([NCH, 1], F32)
                nc.vector.tensor_sub(var_t[:], ex2_t[:], msq_t[:])
                eps_t = gp.tile([NCH, 1], F32)
                nc.gpsimd.memset(eps_t[:], EPS)
                sd_t = gp.tile([NCH, 1], F32)
                nc.scalar.activation(sd_t[:], var_t[:], AF.Sqrt, bias=eps_t[:])
                rstd_t = gp.tile([NCH, 1], F32)
                nc.vector.reciprocal(rstd_t[:], sd_t[:])

                scale_t = gp.tile([NCH, 1], F32)
                nc.vector.tensor_mul(scale_t[:], rstd_t[:], gnw2[:])
                tmp_t = gp.tile([NCH, 1], F32)
                nc.vector.tensor_mul(tmp_t[:], mean_t[:], scale_t[:])
                gb_t = gp.tile([NCH, 1], F32)
                nc.vector.tensor_sub(gb_t[:], gnb2[:], tmp_t[:])
                gb_bf = gp.tile([NCH, 1], BF16)
                nc.scalar.copy(gb_bf[:], gb_t[:])
                scale64_t = gp.tile([NCH, 1], F32)
                nc.scalar.mul(scale64_t[:], scale_t[:], 1.0 / 64.0)

                # folded blkdiag weights for q, k
                blkraw = {}
                for nm in ("q", "k", "v"):
                    b = gp.tile([NCH, NCH], BF16, name=f"blkraw_{nm}")
                    nc.gpsimd.memset(b[:], 0.0)
                    nc.scalar.copy(b[0:C, 0:C], wts[nm][0:C, :])
                    nc.scalar.copy(b[C:NCH, C:NCH], wts[nm][C:NCH, :])
                    blkraw[nm] = b
                blk_p = gp.tile([NCH, NCH], BF16)
                nc.gpsimd.memset(blk_p[:], 0.0)
                nc.scalar.copy(blk_p[0:C, 0:C], wts["p"][0:C, :])
                nc.scalar.copy(blk_p[C:NCH, C:NCH], wts["p"][C:NCH, :])
                nc.scalar.copy(pT64[:], wts["p"][0:C, :])

                for nm in ("q", "k"):
                    wf = gp.tile([NCH, C], F32, name=f"wf_{nm}")
                    nc.vector.tensor_scalar_mul(wf[:], wts[nm][:], scale_t[:])
                    b = pp.tile([NCH, NCH], BF16, name=f"blk_{nm}")
                    nc.gpsimd.memset(b[:], 0.0)
                    nc.scalar.copy(b[0:C, 0:C], wf[0:C, :])
                    nc.scalar.copy(b[C:NCH, C:NCH], wf[C:NCH, :])
                    blk[nm] = b

                btld = {}
                for nm in ("q", "k", "v"):
                    bps = gpp.tile([NCH, 1], F32, tag="bps", bufs=2)
                    nc.tensor.matmul(bps[:], blkraw[nm][:], gb_bf[:],
                                     start=True, stop=True)
                    bt = gp.tile([NCH, 1], F32, name=f"btld_{nm}")
                    nc.scalar.activation(bt[:], bps[:], AF.Identity,
                                         bias=bias_dup[nm][:])
                    btld[nm] = bt
                nc.scalar.copy(btq[:], btld["q"][:])
                nc.scalar.copy(btk[:], btld["k"][:])

                # pv = (P @ Wv~) x + P @ bv~  (proj folded into v path)
                # raw v_w = transpose of host-provided Wv^T
                vwT_bf = gp.tile([C, C], BF16)
                nc.scalar.copy(vwT_bf[:], wts["v"][0:C, :])
                vw_tp = gpp.tile([C, C], BF16, tag="bps", bufs=2)
                nc.tensor.transpose(vw_tp[:], vwT_bf[:], id_bf[0:C, 0:C])
                vw_raw = gp.tile([C, C], BF16)
                nc.vector.tensor_copy(vw_raw[:], vw_tp[:])
                pw_ps = gpp.tile([C, C], F32, tag="bps", bufs=2)
                nc.tensor.matmul(pw_ps[:], vw_raw[:], pT64[:], start=True, stop=True)
                pwf = gp.tile([C, C], F32)
                nc.vector.tensor_scalar_mul(pwf[:], pw_ps[:], scale_t[0:C, :])
                pw_sb = gp.tile([C, C], BF16)
                nc.scalar.copy(pw_sb[:], pwf[:])
                nc.gpsimd.memset(blk_pv[:], 0.0)
                nc.sync.dma_start(blk_pv[0:C, 0:C], pw_sb[:])
                nc.sync.dma_start(blk_pv[C:NCH, C:NCH], pw_sb[:])
                btv_bf = gp.tile([NCH, 1], BF16)
                nc.scalar.copy(btv_bf[:], btld["v"][:])
                bpv_ps = gpp.tile([NCH, 1], F32, tag="bps", bufs=2)
                nc.tensor.matmul(bpv_ps[:], blk_p[:], btv_bf[:], start=True, stop=True)
                nc.scalar.copy(b_pv[:], bpv_ps[:])

                # ---- local qg from local pooled xn ----
                pxn = gp.tile([NCH, 392], BF16)
                nc.vector.tensor_scalar(
                    pxn[:], px[:], scale64_t[:], gb_t[:],
                    op0=mybir.AluOpType.mult, op1=mybir.AluOpType.add,
                )
                pxu = gp.tile([C, PG_LOC], BF16)
                nc.sync.dma_start(pxu[:, 0:392], pxn[0:C, :])
                nc.sync.dma_start(pxu[:, 392:784], pxn[C:NCH, :])
                qb8 = gp.tile([NCH, 1], F32)
                nc.scalar.mul(qb8[:], bias_dup["q"][:], 0.125)
                wT64 = {}
                for nm in ("q", "k", "v"):
                    w64 = gp.tile([C, C], BF16, name=f"w64_{nm}")
                    nc.scalar.copy(w64[:], wts[nm][0:C, :])
                    wT64[nm] = w64
                for s in range(2):
                    cs = slice(s * 392, (s + 1) * 392)
                    qps = gpp.tile([C, 392], F32, tag="pool_ps", bufs=2)
                    nc.tensor.matmul(qps[:], wT64["q"][:], pxu[:, cs],
                                     start=True, stop=True)
                    nc.scalar.activation(qg_bf[:, cs], qps[:], AF.Identity,
                                         bias=qb8[0:C, :], scale=0.125)

                # ---- global kg/vg from gathered raw pooled x ----
                pxg = gp.tile([C, PG], F32)
                nc.sync.dma_start(
                    pxg[:].rearrange("c (r f) -> c r f", r=4),
                    px_out.ap()[:, :, 0:PG_LOC].transpose([1, 0, 2]),
                )
                pxng = gp.tile([C, PG], BF16)
                nc.vector.tensor_scalar(
                    pxng[:], pxg[:], scale64_t[0:C, :], gb_t[0:C, :],
                    op0=mybir.AluOpType.mult, op1=mybir.AluOpType.add,
                )
                for s in range(7):
                    cs = slice(s * 448, (s + 1) * 448)
                    kps = gpp.tile([C, 448], F32, tag="pool_ps", bufs=2)
                    nc.tensor.matmul(kps[:], wT64["k"][:], pxng[:, cs],
                                     start=True, stop=True)
                    nc.scalar.activation(kgf[:, cs], kps[:], AF.Identity,
                                         bias=bias_dup["k"][0:C, :])
                    vps = gpp.tile([C, 448], F32, tag="pool_ps", bufs=2)
                    nc.tensor.matmul(vps[:], wT64["v"][:], pxng[:, cs],
                                     start=True, stop=True)
                    nc.scalar.activation(vgf[:, cs], vps[:], AF.Identity,
                                         bias=bias_dup["v"][0:C, :])

            # vg transposed into [q(part), c|1] chunks; 65th column stays 1.0 so
            # the hg matmul also produces the softmax denominator in row 64
            nc.gpsimd.memset(vgT[:], 1.0)
            with tc.tile_pool(name="vgt_ps", bufs=4, space="PSUM") as vtp:
                for qc in range(25):
                    w = min(128, PG - qc * 128)
                    tp = vtp.tile([128, C], BF16, tag="vgt")
                    nc.tensor.transpose(tp[0:w, :], vgf[:, qc * 128:qc * 128 + w],
                                        id_bf[0:C, 0:C])
                    nc.vector.tensor_copy(vgT[0:w, qc * (C + 1):qc * (C + 1) + C],
                                          tp[0:w, :])

            # ============ phase 2: q/k/pv + patch score accumulation ============
            with tc.tile_pool(name="p2ps", bufs=2, space="PSUM") as p2ps, \
                 tc.tile_pool(name="wmps", bufs=1, space="PSUM") as wmps, \
                 tc.tile_pool(name="p2sb", bufs=3) as p2sb:
                wmA = wmps.tile([128, P2], F32)
                wmB = wmps.tile([68, P2], F32)
                for u in range(64):
                    cols = slice(u * 392, (u + 1) * 392)
                    qps = p2ps.tile([NCH, 392], F32, tag="qps")
                    nc.tensor.matmul(qps[:], blk["q"][:], X[:, cols],
                                     start=True, stop=True)
                    q_sb = p2sb.tile([NCH, 392], BF16, tag="q_sb")
                    nc.scalar.activation(q_sb[:], qps[:], AF.Identity, bias=btq[:])
                    kps = p2ps.tile([NCH, 392], F32, tag="kps")
                    nc.tensor.matmul(kps[:], blk["k"][:], X[:, cols],
                                     start=True, stop=True)
                    k_sb = p2sb.tile([NCH, 392], BF16, tag="k_sb")
                    nc.vector.tensor_scalar_add(k_sb[:], kps[:], btk[:])
                    vps = p2ps.tile([NCH, 392], F32, tag="vps")
                    nc.tensor.matmul(vps[:], blk_pv[:], X[:, cols],
                                     start=True, stop=True)
                    nc.vector.tensor_scalar_add(V[:, cols], vps[:], b_pv[:])
                    for sub in range(2):
                        a = slice(sub * P2, sub * P2 + 128)
                        bsl = slice(sub * P2 + 128, (sub + 1) * P2)
                        ks = k_sb[:, sub * P2:(sub + 1) * P2]
                        first = (u == 0 and sub == 0)
                        last = (u == 63 and sub == 1)
                        nc.tensor.matmul(wmA[:], q_sb[:, a], ks,
                                         start=first, stop=last)
                        nc.tensor.matmul(wmB[:], q_sb[:, bsl], ks,
                                         start=first, stop=last)
                wmA_sb = p2sb.tile([128, P2], F32, tag="wmA_sb", bufs=1)
                nc.vector.tensor_copy(wmA_sb[:], wmA[:])
                wmB_sb = p2sb.tile([68, P2], F32, tag="wmB_sb", bufs=1)
                nc.vector.tensor_copy(wmB_sb[:], wmB[:])
                nc.sync.dma_start(wm_in.ap()[0:128], wmA_sb[:])
                nc.sync.dma_start(wm_in.ap()[128:P2], wmB_sb[:])

            nc.gpsimd.collective_compute(
                "AllReduce", mybir.AluOpType.add, replica_groups=RG,
                ins=[wm_in.ap()], outs=[wm_out.ap()],
            )

            # ============ phase G: global pooled attention ============
            # (emitted here so it overlaps the wm AllReduce latency;
            #  software-pipelined so PE never waits on ACT exp)
            with tc.tile_pool(name="hgps", bufs=1, space="PSUM") as hgp:
                hg_ps = [hgp.tile([C + 1, 392], F32, name=f"hg{s}")
                         for s in range(2)]
                rcp_sb = None
                with tc.tile_pool(name="g1", bufs=4) as g1, \
                     tc.tile_pool(name="g1ps", bufs=6, space="PSUM") as g1ps:
                    e_tiles = {}

                    def emit_sT(qc):
                        w = min(128, PG - qc * 128)
                        e_ch = g1.tile([128, PG_LOC], BF16, tag="e_ch",
                                       name=f"e_{qc}")
                        for s in range(2):
                            cs = slice(s * 392, (s + 1) * 392)
                            sT = g1ps.tile([128, 392], F32, tag="sT")
                            nc.tensor.matmul(sT[0:w, :],
                                             kgf[:, qc * 128:qc * 128 + w],
                                             qg_bf[:, cs], start=True, stop=True)
                            nc.scalar.activation(e_ch[0:w, cs], sT[0:w, :], AF.Exp)
                        e_tiles[qc] = e_ch

                    def emit_acc(qc):
                        w = min(128, PG - qc * 128)
                        e_ch = e_tiles.pop(qc)
                        for s in range(2):
                            cs = slice(s * 392, (s + 1) * 392)
                            nc.tensor.matmul(hg_ps[s][:],
                                             vgT[0:w, qc * (C + 1):
                                                 qc * (C + 1) + C + 1],
                                             e_ch[0:w, cs],
                                             start=(qc == 0), stop=(qc == 24))

                    emit_sT(0)
                    emit_sT(1)
                    for qc in range(25):
                        if qc + 2 < 25:
                            emit_sT(qc + 2)
                        emit_acc(qc)
                    rcp_sb = g1.tile([C + 1, PG_LOC], F32, tag="rcp", bufs=1)
                    nc.vector.reciprocal(rcp_sb[C:C + 1, 0:392],
                                         hg_ps[0][C:C + 1, :])
                    nc.vector.reciprocal(rcp_sb[C:C + 1, 392:784],
                                         hg_ps[1][C:C + 1, :])

                with tc.tile_pool(name="g2", bufs=1) as g2, \
                     tc.tile_pool(name="g2ps", bufs=1, space="PSUM") as g2ps:
                    rcb = g2.tile([C + 1, PG_LOC], BF16)
                    nc.scalar.copy(rcb[C:C + 1, :], rcp_sb[C:C + 1, :])
                    hg_n = g2.tile([C, PG_LOC], BF16)
                    phg_sc = g2.tile([C, PG_LOC], BF16)
                    for s in range(2):
                        cs = slice(s * 392, (s + 1) * 392)
                        rcp128 = g2ps.tile([C, 392], F32, tag="rcp128", bufs=2)
                        nc.tensor.matmul(rcp128[:], ones_bf[C:C + 1, :],
                                         rcb[C:C + 1, cs], start=True, stop=True)
                        rcp128_sb = g2.tile([C, 392], BF16, tag="rcp128_sb", bufs=2)
                        nc.scalar.copy(rcp128_sb[:], rcp128[:])
                        nc.vector.tensor_mul(hg_n[:, cs], hg_ps[s][0:C, :],
                                             rcp128_sb[:])
                    for s in range(2):
                        cs = slice(s * 392, (s + 1) * 392)
                        phg_ps = g2ps.tile([C, 392], F32, tag="phg_ps", bufs=2)
                        nc.tensor.matmul(phg_ps[:], pT64[:], hg_n[:, cs],
                                         start=True, stop=True)
                        nc.vector.tensor_scalar_mul(phg_sc[:, cs], phg_ps[:], 0.25)
                    nc.sync.dma_start(phg[0:C, :], phg_sc[:, 0:392])
                    nc.sync.dma_start(phg[C:NCH, :], phg_sc[:, 392:784])

            # ============ wm softmax + transpose (after AllReduce) ============
            with tc.tile_pool(name="wmsb", bufs=1) as wsb, \
                 tc.tile_pool(name="wmtp", bufs=4, space="PSUM") as wtp:
                wmn = {}
                for nm, rows in (("hi", slice(0, 128)), ("lo", slice(128, P2))):
                    n = rows.stop - rows.start
                    wt = wsb.tile([n, P2], F32, name=f"wmr_{nm}")
                    nc.sync.dma_start(wt[:], wm_out.ap()[rows])
                    mx = wsb.tile([n, 1], F32, name=f"mx_{nm}")
                    nc.vector.reduce_max(mx[:], wt[:], axis=AXX)
                    mxn = wsb.tile([n, 1], F32, name=f"mxn_{nm}")
                    nc.vector.tensor_scalar_mul(mxn[:], mx[:], -1.0 / 256.0)
                    we = wsb.tile([n, P2], F32, name=f"we_{nm}")
                    rs = wsb.tile([n, 1], F32, name=f"rs_{nm}")
                    nc.scalar.activation(we[:], wt[:], AF.Exp, bias=mxn[:],
                                         scale=1.0 / 256.0, accum_out=rs[:])
                    rc = wsb.tile([n, 1], F32, name=f"rc_{nm}")
                    nc.vector.reciprocal(rc[:], rs[:])
                    rc75 = wsb.tile([n, 1], F32, name=f"rc75_{nm}")
                    nc.scalar.mul(rc75[:], rc[:], 0.75)
                    wn = wsb.tile([n, P2], BF16, name=f"wmn_{nm}")
                    nc.vector.tensor_scalar_mul(wn[:], we[:], rc75[:])
                    wmn[nm] = wn
                for src, csl, dst, dcols in (
                    ("hi", slice(0, 128), wmT_hi, slice(0, 128)),
                    ("hi", slice(128, P2), wmT_lo, slice(0, 128)),
                    ("lo", slice(0, 128), wmT_hi, slice(128, P2)),
                    ("lo", slice(128, P2), wmT_lo, slice(128, P2)),
                ):
                    inap = wmn[src][:, csl]
                    n_p = inap.shape[0]
                    n_k = csl.stop - csl.start
                    tp = wtp.tile([128, 128], BF16, tag="wmt_ps")
                    nc.tensor.transpose(tp[0:n_k, 0:n_p], inap, id_bf[0:n_p, 0:n_p])
                    nc.vector.tensor_copy(dst[0:n_k, dcols], tp[0:n_k, 0:n_p])

            # ====== phase 3: patch attention (+proj via pv) + residual ======
            # ====== phase 4 (interleaved): add upsampled global, store ======
            # Software-pipelined: transposes for chunk t+1 are emitted before
            # the matmuls of chunk t so PE always has ready work.
            with tc.tile_pool(name="p3ps", bufs=3, space="PSUM") as p3ps, \
                 tc.tile_pool(name="p3sb", bufs=4) as p3sb:
                vt_tiles = {}

                def emit_tr(t):
                    ta = p3ps.tile([128, 128], BF16, tag="ta")
                    nc.tensor.transpose(ta[:], V[:, t * P2:t * P2 + 128], id_bf[:])
                    tb = p3ps.tile([68, 128], BF16, tag="tb")
                    nc.tensor.transpose(tb[:], V[:, t * P2 + 128:(t + 1) * P2],
                                        id_bf[:])
                    vta = p3sb.tile([128, 128], BF16, tag="vta")
                    nc.vector.tensor_copy(vta[:], ta[:])
                    vtb = p3sb.tile([68, 128], BF16, tag="vtb")
                    nc.scalar.copy(vtb[:], tb[:])
                    vt_tiles[t] = (vta, vtb)

                def emit_mm(t):
                    tcols = slice(t * P2, (t + 1) * P2)
                    vta, vtb = vt_tiles.pop(t)
                    hp_ps = p3ps.tile([128, P2], F32, tag="hp_ps", bufs=2)
                    nc.tensor.matmul(hp_ps[:], vta[:], wmT_hi[:],
                                     start=True, stop=False)
                    nc.tensor.matmul(hp_ps[:], vtb[:], wmT_lo[:],
                                     start=False, stop=True)
                    nc.vector.tensor_add(V[:, tcols], hp_ps[:], X[:, tcols])

                emit_tr(0)
                emit_tr(1)
                for t in range(128):
                    if t + 2 < 128:
                        emit_tr(t + 2)
                    emit_mm(t)
                    ci = T4.get(t)
                    if ci is not None:
                        cols = slice(ci * 3584, (ci + 1) * 3584)
                        bcast = (phg[:, ci * 56:(ci + 1) * 56]
                                 .unsqueeze(1).unsqueeze(3)
                                 .broadcast_to([NCH, 8, 56, 8]))
                        v4 = V[:, cols].rearrange("p (r pc cc) -> p r pc cc",
                                                  r=8, pc=56, cc=8)
                        nc.gpsimd.tensor_tensor(v4, v4, bcast,
                                                op=mybir.AluOpType.add)
                        eng = nc.sync if ci % 2 == 0 else nc.scalar
                        eng.dma_start(out_t.ap()[:, cols], V[:, cols])

    nc.compile()
    return nc


_NC = None


def _get_nc():
    global _NC
    if _NC is None:
        _NC = build()
    return _NC


def _pack_x(x):
    import ml_dtypes
    # [2,64,448,448] -> [8 cores, 128, 25088] stacked bf16
    xs = x.reshape(2, C, 4, 2, 56, SIZE).transpose(0, 2, 3, 1, 4, 5)
    return np.ascontiguousarray(xs.reshape(8, NCH, HALF)).astype(ml_dtypes.bfloat16)


def _unpack_out(o):
    # [8, 128, 25088] bf16 -> [2, 64, 448, 448] f32
    o = np.asarray(o, dtype=np.float32).reshape(2, 4, 2, C, 56, SIZE)
    return np.ascontiguousarray(o.transpose(0, 3, 1, 2, 4, 5)).reshape(
        2, C, SIZE, SIZE)


def kernel(x, gn_w, gn_b, q_w, q_b, k_w, k_b, v_w, v_b, proj_w):
    nc = _get_nc()
    x = np.asarray(x, dtype=np.float32)
    xp = _pack_x(x)
    small = {
        "gn_w": gn_w, "gn_b": gn_b, "q_b": q_b, "k_b": k_b, "v_b": v_b,
    }
    small = {k: np.ascontiguousarray(np.asarray(v, dtype=np.float32))
             for k, v in small.items()}
    # weights are consumed pre-transposed on device
    for nm, w in (("q_w", q_w), ("k_w", k_w), ("v_w", v_w), ("proj_w", proj_w)):
        small[nm] = np.ascontiguousarray(np.asarray(w, dtype=np.float32).T)
    in_maps = []
    for r in range(8):
        m = dict(small)
        m["x"] = xp[r]
        in_maps.append(m)
    res = bass_utils.run_bass_kernel_spmd(nc, in_maps, core_ids=list(range(8)))
    return _unpack_out(np.stack([res.results[r]["out"] for r in range(8)]))


if __name__ == "__main__":
    build()
    print("build+compile OK")
